# revision 8
# baseline (speedup 1.0000x reference)
"""Causal multi-head attention on 8 Trainium2 NeuronCores.

Tensor-parallel over heads: 16 heads -> 2 heads per core (128 of the 1024
model dims per core). Each core computes q/k/v projections for its head
slice, causal attention, and its partial output projection (row-slice of
Wo); the host sums the 8 partials (+bo).

All matmuls run bf16 (pipelined LDWEIGHTS, full PE rate). Per-core layouts
(partition dim first):
  xT     [1024, 8192] bf16  x transposed (host-prepared)
  qT/kT  [128, 2048]/b      head dims on partitions (h0: 0-63, h1: 64-127)
  vplus  [128, 16, 130]/b   [v_h0(64) | ones | v_h1(64) | ones] per key tile
  scores sT = k @ qT        [128 keys, q] x 2 heads, row-packed concurrent MMs
  ctxT   [65, 512] psum     rows 0-63 = unnormalized ctx^T, row 64 = denom
Diagonal key tiles only compute/exp/accumulate the q-columns at or right of
the diagonal; they are processed FIRST within a q-chunk (descending offset)
so the accumulation group starts on the diagonal and stops on a full-width
tile. The attention kt-loop pulls "filler" units (next batch's projection
chains, previous chunk's out-projection) between steps so the statically
scheduled tensor queue stays busy during scalar-engine exp latency.
"""

import numpy as np
from collections import deque
from contextlib import ExitStack

import concourse.bass as bass
import concourse.mybir as mybir
import concourse.tile as tile
from concourse import bacc
from concourse import bass_utils
from concourse.masks import make_identity

F32 = mybir.dt.float32
BF16 = mybir.dt.bfloat16
AF = mybir.ActivationFunctionType

B, S, D = 4, 2048, 1024
H, DH = 16, 64
NCORES = 8
DHC = 128           # head dims per core (2 heads x 64)
BS = B * S          # 8192
QC = 512            # q-chunk width
NQC = S // QC       # 4 q-chunks per batch
NKT = S // 128      # 16 key tiles per batch
NKD = D // 128      # 8 contraction tiles for projections

_CACHE = {}


def _build():
    nc = bacc.Bacc("TRN2", target_bir_lowering=False, debug=False)
    xT = nc.dram_tensor("xT", [D, BS], BF16, kind="ExternalInput").ap()
    wqkv = nc.dram_tensor("wqkv", [D, 3 * DHC], BF16, kind="ExternalInput").ap()
    bqkv = nc.dram_tensor("bqkv", [DHC, 3], F32, kind="ExternalInput").ap()
    wo = nc.dram_tensor("wo", [DHC, D], BF16, kind="ExternalInput").ap()
    cmask = nc.dram_tensor("cmask", [128, 128], BF16, kind="ExternalInput").ap()
    out = nc.dram_tensor("out", [BS, D], BF16, kind="ExternalOutput").ap()

    with tile.TileContext(nc) as tc:
        with ExitStack() as ctx:
            consts = ctx.enter_context(tc.tile_pool(name="consts", bufs=1))
            big = ctx.enter_context(tc.tile_pool(name="big", bufs=2))
            bigc = ctx.enter_context(tc.tile_pool(name="bigc", bufs=2))
            work = ctx.enter_context(tc.tile_pool(name="work", bufs=3))
            expp = ctx.enter_context(tc.tile_pool(name="expp", bufs=6))
            outp = ctx.enter_context(tc.tile_pool(name="outp", bufs=3))
            small = ctx.enter_context(tc.tile_pool(name="small", bufs=4))
            psum_c = ctx.enter_context(tc.tile_pool(name="psum_c", bufs=1, space="PSUM"))
            psum_s = ctx.enter_context(tc.tile_pool(name="psum_s", bufs=2, space="PSUM"))
            psum_w = ctx.enter_context(tc.tile_pool(name="psum_w", bufs=2, space="PSUM"))

            # ---- constants (weights first: first proj chain needs them) ----
            t_w = consts.tile([128, NKD, 3 * DHC], BF16, tag="w")
            for kd in range(NKD):
                nc.gpsimd.dma_start(
                    t_w[:, kd, :], wqkv[kd * 128:(kd + 1) * 128, :]
                )
            t_bqkv = consts.tile([DHC, 3], F32, tag="bqkv")
            nc.gpsimd.dma_start(t_bqkv, bqkv)
            t_wo = consts.tile([DHC, D], BF16, tag="wo")
            nc.gpsimd.dma_start(t_wo, wo)
            t_mask = consts.tile([128, 128], BF16, tag="mask")
            nc.gpsimd.dma_start(t_mask, cmask)
            t_idf = consts.tile([128, 128], F32, tag="idf")
            make_identity(nc, t_idf)
            t_id = consts.tile([128, 128], BF16, tag="id")
            nc.vector.tensor_copy(t_id, t_idf)
            t_ones = consts.tile([128, 1], F32, tag="ones")
            nc.vector.memset(t_ones, 1.0)

            qT = {}
            kT = {}
            vplus = {}
            ctxT = {}
            xt_tiles = {}

            def unit_dma_xt(b, qc):
                if qc == 0:
                    qT[b] = big.tile([128, S], BF16, tag="qT", name=f"qT{b}")
                    kT[b] = big.tile([128, S], BF16, tag="kT", name=f"kT{b}")
                    vplus[b] = big.tile([128, NKT, 130], BF16, tag="vplus",
                                        name=f"vplus{b}")
                    nc.vector.tensor_copy(
                        vplus[b][:, :, 64:65],
                        t_ones[:, None, :].broadcast_to([128, NKT, 1]),
                    )
                    nc.vector.tensor_copy(
                        vplus[b][:, :, 129:130],
                        t_ones[:, None, :].broadcast_to([128, NKT, 1]),
                    )
                j0 = b * S + qc * QC
                xt = work.tile([128, NKD, QC], BF16, tag="xt", name="xt")
                nc.sync.dma_start(
                    xt, xT.rearrange("(t p) c -> p t c", p=128)[:, :, j0:j0 + QC]
                )
                xt_tiles[(b, qc)] = xt

            def unit_proj_chain(b, qc, pi):
                """One projection output chain: 8 accumulating MMs + bias."""
                xt = xt_tiles[(b, qc)]
                ps = psum_w.tile([128, QC], F32, tag="pp", name="ps_proj")
                for kd in range(NKD):
                    nc.tensor.matmul(
                        ps,
                        t_w[:, kd, pi * DHC:(pi + 1) * DHC],
                        xt[:, kd, :],
                        start=(kd == 0),
                        stop=(kd == NKD - 1),
                    )
                bias_ap = t_bqkv[:, pi:pi + 1]
                if pi == 0:
                    nc.vector.tensor_scalar_add(
                        qT[b][:, qc * QC:(qc + 1) * QC], ps, bias_ap
                    )
                elif pi == 1:
                    nc.vector.tensor_scalar_add(
                        kT[b][:, qc * QC:(qc + 1) * QC], ps, bias_ap
                    )
                else:
                    vst = small.tile([128, QC], BF16, tag="vstage")
                    nc.vector.tensor_scalar_add(vst, ps, bias_ap)
                    for tt in range(4):
                        loc = qc * 4 + tt  # key-tile index within batch
                        ps_t = psum_w.tile([128, 128], BF16, tag="pp", name="ps_t")
                        nc.tensor.transpose(
                            ps_t, vst[:, tt * 128:(tt + 1) * 128], t_id
                        )
                        # h0 dims -> cols 0:64, h1 dims -> cols 65:129
                        nc.any.tensor_copy(
                            vplus[b][:, loc, :].rearrange(
                                "p (g c) -> p g c", g=2)[:, :, 0:64],
                            ps_t.rearrange("p (g c) -> p g c", g=2),
                        )

            def unit_outproj(b, qc, qi):
                qt = qc * 4 + qi
                r0 = b * S + qt * 128
                t_o = outp.tile([128, D], BF16, tag="out", name="t_o")
                for ch in range(2):
                    ps_o = psum_w.tile([128, QC], F32, tag="pp", name="ps_o")
                    nc.tensor.matmul(
                        ps_o,
                        ctxT[b][:, qt * 128:(qt + 1) * 128],
                        t_wo[:, ch * QC:(ch + 1) * QC],
                        start=True, stop=True,
                    )
                    if ch == 0:
                        nc.vector.tensor_copy(t_o[:, 0:QC], ps_o)
                    else:
                        nc.scalar.copy(t_o[:, QC:D], ps_o)
                nc.sync.dma_start(out[r0:r0 + 128, :], t_o)

            def proj_units(b):
                for qc in range(NQC):
                    yield lambda qc=qc: unit_dma_xt(b, qc)
                    for pi in range(3):
                        yield lambda qc=qc, pi=pi: unit_proj_chain(b, qc, pi)

            fill_q = deque()

            def pull_filler(n=1):
                for _ in range(n):
                    if not fill_q:
                        return
                    gen = fill_q[0]
                    try:
                        u = next(gen)
                        if callable(u):
                            u()
                    except StopIteration:
                        fill_q.popleft()

            def emit_attn_qc(b, qc):
                """Causal attention for batch b, q-chunk qc; pulls fillers."""
                if qc == 0:
                    ctxT[b] = bigc.tile([128, S], BF16, tag="ctxT", name=f"ctxT{b}")
                q0 = qc * QC
                ps_c0 = psum_c.tile([65, QC], F32, tag="ctx0")
                ps_c1 = psum_c.tile([65, QC], F32, tag="ctx1")
                nkt = 4 * qc + 4
                # diagonal tiles first (descending offset), then full tiles:
                # the accumulation group starts on the narrowest diagonal
                # write and stops on a final full-width tile.
                kts = [4 * qc + o for o in (3, 2, 1, 0)] + list(range(4 * qc))
                ps_list = [None] * nkt

                def emit_scores(i):
                    kt = kts[i]
                    o = kt - 4 * qc
                    qoff = o * 128 if o > 0 else 0
                    ps_s = psum_s.tile([128, 2 * QC], F32, tag="scores")
                    nc.tensor.matmul(
                        ps_s[:, qoff:QC],
                        kT[b][0:64, kt * 128:(kt + 1) * 128],
                        qT[b][0:64, q0 + qoff:q0 + QC],
                        start=True, stop=True,
                    )
                    nc.tensor.matmul(
                        ps_s[:, QC + qoff:2 * QC],
                        kT[b][64:128, kt * 128:(kt + 1) * 128],
                        qT[b][64:128, q0 + qoff:q0 + QC],
                        start=True, stop=True,
                        tile_position=(64, 0),
                    )
                    ps_list[i] = (ps_s, qoff)

                emit_scores(0)
                for i, kt in enumerate(kts):
                    o = kt - 4 * qc  # diagonal offset (>=0 on diagonal tiles)
                    ps_s, qoff = ps_list[i]
                    first = i == 0
                    last = i == nkt - 1
                    t_exp = expp.tile([128, 2, QC], BF16, tag="exp")
                    nc.scalar.activation(
                        t_exp[:, :, qoff:QC],
                        ps_s.rearrange("p (g c) -> p g c", g=2)[:, :, qoff:QC],
                        AF.Exp, scale=0.125,
                    )
                    # while exp(i) runs on ScalarE, keep TensorE busy with
                    # the next scores pair and one filler unit
                    if i + 1 < nkt:
                        emit_scores(i + 1)
                    pull_filler(1)
                    if o >= 0:  # causal strip: zero the masked triangle
                        nc.vector.tensor_mul(
                            t_exp[:, :, qoff:qoff + 128],
                            t_exp[:, :, qoff:qoff + 128],
                            t_mask[:, None, :].broadcast_to([128, 2, 128]),
                        )
                    nc.tensor.matmul(
                        ps_c0[:, qoff:QC], vplus[b][:, kt, 0:65],
                        t_exp[:, 0, qoff:QC],
                        start=first, stop=last,
                    )
                    nc.tensor.matmul(
                        ps_c1[:, qoff:QC], vplus[b][:, kt, 65:130],
                        t_exp[:, 1, qoff:QC],
                        start=first, stop=last,
                    )
                # softmax normalization (denominator in row 64)
                for h, ps_c in ((0, ps_c0), (1, ps_c1)):
                    t_d = small.tile([1, QC], F32, tag="den")
                    nc.vector.tensor_copy(t_d, ps_c[64:65, :])
                    t_r = small.tile([1, QC], F32, tag="recip")
                    nc.vector.reciprocal_approx_fast(t_r, t_d)
                    t_bc = small.tile([64, QC], F32, tag="bcast")
                    nc.gpsimd.partition_broadcast(t_bc, t_r)
                    nc.vector.tensor_mul(
                        ctxT[b][h * 64:(h + 1) * 64, q0:q0 + QC],
                        ps_c[0:64, :],
                        t_bc,
                    )
                # out-projection of this chunk becomes filler work
                def op_units(b=b, qc=qc):
                    for qi in range(QC // 128):
                        yield unit_outproj(b, qc, qi)
                fill_q.append(op_units())

            # ---- top-level emission ----
            for gen in proj_units(0):
                gen()
            fill_q.append(proj_units(1))
            for b in range(B):
                for qc in range(NQC):
                    emit_attn_qc(b, qc)
                    pull_filler(2)
                if b + 2 < B:
                    fill_q.append(proj_units(b + 2))
            while fill_q:
                pull_filler(1)

    nc.compile()
    return nc


def _host_inputs(x, wq, bq, wk, bk, wv, bv, wo, bo):
    import ml_dtypes
    bf16 = ml_dtypes.bfloat16
    x = np.asarray(x, dtype=np.float32).reshape(BS, D)
    xT = np.ascontiguousarray(x.T).astype(bf16)
    # causal mask for the 128-wide diagonal strip: keep j >= p
    p = np.arange(128)[:, None]
    j = np.arange(128)[None, :]
    cmask = (j >= p).astype(np.float32).astype(bf16)
    wq, wk, wv, wo = (np.asarray(a, dtype=np.float32) for a in (wq, wk, wv, wo))
    bq, bk, bv, bo = (np.asarray(a, dtype=np.float32) for a in (bq, bk, bv, bo))
    in_maps = []
    for c in range(NCORES):
        sl = slice(c * DHC, (c + 1) * DHC)
        wqkv = np.ascontiguousarray(
            np.concatenate([wq[:, sl], wk[:, sl], wv[:, sl]], axis=1)
        ).astype(bf16)
        bqkv = np.ascontiguousarray(np.stack([bq[sl], bk[sl], bv[sl]], axis=1))
        in_maps.append({
            "xT": xT,
            "wqkv": wqkv,
            "bqkv": bqkv,
            "wo": np.ascontiguousarray(wo[sl, :]).astype(bf16),
            "cmask": cmask,
        })
    return in_maps


def kernel(x, wq, bq, wk, bk, wv, bv, wo, bo, _trace=False, _tmpdir=None):
    if "nc" not in _CACHE:
        _CACHE["nc"] = _build()
    nc = _CACHE["nc"]
    in_maps = _host_inputs(x, wq, bq, wk, bk, wv, bv, wo, bo)
    res = bass_utils.run_bass_kernel_spmd(
        nc, in_maps, core_ids=list(range(NCORES)), trace=_trace, tmpdir=_tmpdir
    )
    _CACHE["last_results"] = res
    acc = np.zeros((BS, D), dtype=np.float64)
    for c in range(NCORES):
        acc += np.asarray(res.results[c]["out"], dtype=np.float64)
    acc += np.asarray(bo, dtype=np.float64)[None, :]
    return acc.astype(np.float32).reshape(B, S, D)
